# revision 9
# baseline (speedup 1.0000x reference)
"""Trainium2 Bass kernel for nn_EntityEncoder (multi-hot embedding bag + MLP head).

Strategy: vocab (E) sharding across 8 cores. The host lays out each core's
[512, 6250] mask slice in transposed, zero-padded, SBUF-ready bf16 form
[128, 49*512] (partition = e-within-subtile, free = subtile-major bp), and the
matching embedding rows as bf16 [128, 49*128]. Each core:
  - DMAs its 6.4 MB mask slice in 9 chunks alternating between two DMA queues
    (sync/gpsimd) and the 1.6 MB embedding slice on the scalar queue; both are
    fully SBUF-resident,
  - accumulates sums[h, bp] over 49 K=128 subtiles as bf16 matmuls straight
    from the resident tiles,
  - computes counts with within-chunk pair-add trees on DVE (exact small-int
    bf16 sums) plus a few ones-matmuls,
  - exchanges partials with a single 129-row bf16 ReduceScatter (128 sums rows
    + 1 bf16 count row) so each core receives the fully-reduced block for its
    own 64 paths,
  - computes the head (divide, path-mean, LN -> Linear+ReLU -> BN, x2) for its
    own 4 batches, using gpsimd partition_broadcast for row broadcasts and
    bf16 weights for the two linears; output is [H, 4], host transposes.
The first-collective bootstrap barrier starts at NEFF begin and overlaps the
compute phase; the RS is the only collective.
LN gamma/beta are folded into the following linear's weights on the host.
"""

import numpy as np

B, P, E, H = 32, 16, 50000, 128
NCORES = 8
BP = B * P                 # 512
E_SH = E // NCORES         # 6250 vocab rows per core
SUB = 128                  # matmul K subtile
NSUB = 49                  # ceil(6250/128)
E_PAD = NSUB * SUB         # 6272
CHUNKS = [1, 8, 8, 8, 8, 8, 8]  # subtiles per DMA chunk (sum = 49)
EPS = 1e-5
NB = BP // NCORES          # 64 paths per core after ReduceScatter
BL = B // NCORES           # 4 local batches

# packed params layout: par (f32) [128, 10]:
#  col 4 bn1_g', 5 bn1_b, 6 bn2_g', 7 bn2_b, 8 b1', 9 b2'
# par_w (bf16) [128, 256]: 0:128 (w1*ln1_g)^T, 128:256 (w2*ln2_g)^T
NPAR = 10

_cached = {}


def _build():
    import concourse.bacc as bacc
    import concourse.mybir as mybir
    import concourse.tile as tile

    f32 = mybir.dt.float32
    bf16 = mybir.dt.bfloat16

    nc = bacc.Bacc("TRN2", target_bir_lowering=False, debug=False,
                   num_devices=NCORES)

    x_d = nc.dram_tensor("x", [SUB, NSUB * BP], bf16, kind="ExternalInput")
    emb_d = nc.dram_tensor("emb", [SUB, NSUB * H], bf16, kind="ExternalInput")
    par_d = nc.dram_tensor("par", [128, NPAR], f32, kind="ExternalInput")
    parw_d = nc.dram_tensor("parw", [128, 256], bf16, kind="ExternalInput")
    out_d = nc.dram_tensor("out", [H, BL], f32, kind="ExternalOutput")

    with tile.TileContext(nc) as tc:
        with tc.tile_pool(name="const", bufs=1) as constp, \
             tc.tile_pool(name="head", bufs=1) as head, \
             tc.tile_pool(name="ps_acc", bufs=1, space="PSUM") as ps_acc, \
             tc.tile_pool(name="ps_misc", bufs=3, space="PSUM") as ps_misc, \
             tc.tile_pool(name="dram", bufs=1, space="DRAM") as dram:

            ones_col = constp.tile([128, 1], f32)
            nc.vector.memset(ones_col[:], 1.0)
            zero_1 = constp.tile([1, 1], f32)
            nc.vector.memset(zero_1[:], 0.0)
            ones_bf = constp.tile([128, 1], bf16)
            nc.vector.memset(ones_bf[:], 1.0)

            par = constp.tile([128, NPAR], f32)
            nc.scalar.dma_start(par[:], par_d[:, :])
            par_w = constp.tile([128, 256], bf16)
            nc.scalar.dma_start(par_w[:], parw_d[:, :])

            # preload the Sqrt ACT table so the head doesn't pay the
            # 1.3us table load on the critical path
            warm = constp.tile([1, 1], f32)
            nc.scalar.activation(warm[:], zero_1[:],
                                 mybir.ActivationFunctionType.Sqrt,
                                 bias=zero_1[:, :1], scale=1.0)
            warm2 = constp.tile([1, 1], f32)
            nc.vector.reciprocal(warm2[:], ones_col[0:1, :])
            warm3 = constp.tile([128, 1], f32)
            nc.gpsimd.partition_broadcast(warm3[:], warm2[:])

            # resident input tiles
            xr = constp.tile([SUB, NSUB * BP], bf16)
            emb_b = constp.tile([SUB, NSUB * H], bf16)
            # per-chunk count partials (chunks 1..8; chunk 0 aliases xr)
            cres = constp.tile([SUB, 8 * BP], bf16)

            psum_sums = ps_acc.tile([128, BP], f32)   # [h, bp]
            psum_cnt = ps_acc.tile([1, BP], f32)

            n_cnt_mm = 4  # pairs (0,1),(2,3),(4,5),(6)
            cnt_mm_done = 0
            chunk_res = []  # AP getters for per-chunk count partials
            sidx = 0
            for t, S in enumerate(CHUNKS):
                lo, hi = sidx * BP, (sidx + S) * BP
                qeng = (nc.sync, nc.gpsimd, nc.sync, nc.gpsimd,
                        nc.scalar, nc.gpsimd, nc.sync)[t]
                qeng.dma_start(xr[:, lo:hi], x_d[:, lo:hi])
                elo, ehi = sidx * H, (sidx + S) * H
                nc.scalar.dma_start(emb_b[:, elo:ehi], emb_d[:, elo:ehi])
                for j in range(S):
                    g = sidx + j
                    nc.tensor.matmul(
                        psum_sums[:],
                        emb_b[:, g * H:(g + 1) * H],
                        xr[:, g * BP:(g + 1) * BP],
                        start=(g == 0), stop=(g == NSUB - 1))
                # within-chunk count tree: sequential adds into cres slice
                # (exact in bf16: integer sums of 0/1 masks, max 12 < 256)
                if S == 1:
                    chunk_res.append(xr[:, lo:hi])
                else:
                    cr = cres[:, (t - 1) * BP:t * BP]
                    nc.vector.tensor_tensor(
                        out=cr, in0=xr[:, lo:lo + BP],
                        in1=xr[:, lo + BP:lo + 2 * BP],
                        op=mybir.AluOpType.add)
                    for j in range(2, S):
                        nc.vector.tensor_tensor(
                            out=cr, in0=cr,
                            in1=xr[:, lo + j * BP:lo + (j + 1) * BP],
                            op=mybir.AluOpType.add)
                    chunk_res.append(cr)
                # chunk-pair combine + ones-matmul into psum_cnt
                if t % 2 == 1 or t == len(CHUNKS) - 1:
                    if t % 2 == 1:
                        nc.vector.tensor_tensor(
                            out=chunk_res[t], in0=chunk_res[t],
                            in1=chunk_res[t - 1], op=mybir.AluOpType.add)
                    nc.tensor.matmul(
                        psum_cnt[:], ones_bf[:, :], chunk_res[t],
                        start=(cnt_mm_done == 0),
                        stop=(cnt_mm_done == n_cnt_mm - 1))
                    cnt_mm_done += 1
                sidx += S

            # ------------- single merged ReduceScatter ----------------
            # 8 blocks of 129 rows: 128 bf16 sums rows + 1 bf16 count row
            sums_stage = head.tile([128, BP], bf16)
            nc.scalar.copy(sums_stage[:], psum_sums[:])
            cnt_bf = head.tile([1, BP], bf16)
            with nc.allow_low_precision(reason="counts ~25k, bf16 rel 0.4%"):
                nc.vector.tensor_copy(cnt_bf[:], psum_cnt[:])

            ccs_in = dram.tile([NCORES * 129, NB], bf16)
            ccs_out = dram.tile([129, NB], bf16)
            ccs_v = ccs_in[:].rearrange("(s r) c -> r s c", r=129)
            nc.scalar.dma_start(
                ccs_v[0:128],
                sums_stage[:].rearrange("p (s c) -> p s c", c=NB))
            nc.sync.dma_start(
                ccs_v[128:129],
                cnt_bf[:].rearrange("p (s c) -> p s c", c=NB))
            nc.gpsimd.collective_compute(
                "ReduceScatter",
                mybir.AluOpType.add,
                replica_groups=[list(range(NCORES))],
                ins=[ccs_in[:].opt()],
                outs=[ccs_out[:].opt()],
            )
            sums_loc = head.tile([128, NB], bf16)
            nc.scalar.dma_start(sums_loc[:], ccs_out[0:128, :])
            cnt_row = head.tile([1, NB], bf16)
            nc.sync.dma_start(cnt_row[:], ccs_out[128:129, :])

            # ---------------- head (local 64 paths / 4 batches) -------
            rec = head.tile([1, NB], f32)
            nc.vector.reciprocal(rec[:], cnt_row[:])
            rec_bc = head.tile([128, NB], f32)
            nc.gpsimd.partition_broadcast(rec_bc[:], rec[:])
            path = head.tile([128, NB], f32)
            nc.vector.tensor_tensor(out=path[:], in0=sums_loc[:],
                                    in1=rec_bc[:], op=mybir.AluOpType.mult)
            x0 = head.tile([128, BL], f32)
            nc.vector.reduce_sum(
                x0[:], path[:].rearrange("h (b p) -> h b p", p=P),
                axis=mybir.AxisListType.X)

            def layer_norm(x_sb, eps_val, name):
                sq = head.tile([128, BL], f32, tag=f"{name}_sq")
                nc.vector.tensor_tensor(out=sq[:], in0=x_sb[:], in1=x_sb[:],
                                        op=mybir.AluOpType.mult)
                st_ps = ps_misc.tile([1, 2 * BL], f32, tag="psmisc")
                nc.tensor.matmul(st_ps[:, 0:BL], ones_col[:], x_sb[:],
                                 start=True, stop=True)
                nc.tensor.matmul(st_ps[:, BL:2 * BL], ones_col[:], sq[:],
                                 start=True, stop=True)
                # mu = Sx/128 ; var+eps = (Sx2/128 + eps) - mu^2
                mr = head.tile([1, 2 * BL], f32, tag=f"{name}_mr")
                nc.vector.tensor_scalar(
                    out=mr[:, 0:BL], in0=st_ps[:, 0:BL],
                    scalar1=1.0 / 128, scalar2=None,
                    op0=mybir.AluOpType.mult)
                mu2 = head.tile([1, BL], f32, tag=f"{name}_mu2")
                nc.vector.tensor_tensor(
                    out=mu2[:], in0=mr[:, 0:BL], in1=mr[:, 0:BL],
                    op=mybir.AluOpType.mult)
                var = head.tile([1, BL], f32, tag=f"{name}_var")
                nc.vector.tensor_scalar(
                    out=var[:], in0=st_ps[:, BL:2 * BL],
                    scalar1=1.0 / 128, scalar2=float(eps_val),
                    op0=mybir.AluOpType.mult, op1=mybir.AluOpType.add)
                nc.vector.tensor_tensor(
                    out=var[:], in0=var[:], in1=mu2[:],
                    op=mybir.AluOpType.subtract)
                sd = head.tile([1, BL], f32, tag=f"{name}_sd")
                nc.scalar.activation(sd[:], var[:],
                                     mybir.ActivationFunctionType.Sqrt,
                                     bias=zero_1[:, :1], scale=1.0)
                nc.vector.reciprocal(mr[:, BL:2 * BL], sd[:])
                bcs = head.tile([128, 2 * BL], f32, tag=f"{name}_bcs")
                nc.gpsimd.partition_broadcast(bcs[:], mr[:])
                xn = head.tile([128, BL], f32, tag=f"{name}_xn")
                nc.vector.tensor_tensor(
                    out=xn[:], in0=x_sb[:], in1=bcs[:, 0:BL],
                    op=mybir.AluOpType.subtract)
                xnb = head.tile([128, BL], bf16, tag=f"{name}_xnb")
                with nc.allow_low_precision(reason="bf16 matmul input"):
                    nc.vector.tensor_tensor(
                        out=xnb[:], in0=xn[:], in1=bcs[:, BL:2 * BL],
                        op=mybir.AluOpType.mult)
                return xnb

            def linear_relu_bn(x_bf, w_lo, b_col, bng_col, bnb_col, name):
                y_ps = ps_misc.tile([128, BL], f32, tag="psmisc")
                nc.tensor.matmul(y_ps[:], par_w[:, w_lo:w_lo + 128], x_bf[:],
                                 start=True, stop=True)
                y = head.tile([128, BL], f32, tag=f"{name}_relu")
                nc.vector.tensor_scalar(
                    out=y[:], in0=y_ps[:],
                    scalar1=par[:, b_col:b_col + 1], scalar2=0.0,
                    op0=mybir.AluOpType.add, op1=mybir.AluOpType.max)
                z = head.tile([128, BL], f32, tag=f"{name}_bn")
                nc.vector.tensor_scalar(
                    out=z[:], in0=y[:],
                    scalar1=par[:, bng_col:bng_col + 1],
                    scalar2=par[:, bnb_col:bnb_col + 1],
                    op0=mybir.AluOpType.mult, op1=mybir.AluOpType.add)
                return z

            # LN1 on un-normalized p-sum: eps scales by P^2
            h1 = layer_norm(x0, EPS * P * P, "ln1")
            h2 = linear_relu_bn(h1, 0, 8, 4, 5, "l1")
            h3 = layer_norm(h2, EPS, "ln2")
            h4 = linear_relu_bn(h3, 128, 9, 6, 7, "l2")

            # store [128h, 4b]; host transposes
            nc.scalar.dma_start(out_d[:, :], h4[:])

    nc.compile()
    return nc


def _prepare_in_maps(inputs):
    import ml_dtypes
    bf16 = ml_dtypes.bfloat16

    x = np.asarray(inputs["inputs"])
    emb = np.asarray(inputs["emb"], dtype=np.float32)
    w1 = np.asarray(inputs["w1"], dtype=np.float32)
    b1 = np.asarray(inputs["b1"], dtype=np.float32)
    w2 = np.asarray(inputs["w2"], dtype=np.float32)
    b2 = np.asarray(inputs["b2"], dtype=np.float32)

    par = np.zeros((128, NPAR), dtype=np.float32)
    par[:, 4] = np.asarray(inputs["bn1_g"], np.float32) / np.sqrt(
        np.float32(1.0) + np.float32(EPS))
    par[:, 5] = inputs["bn1_b"]
    par[:, 6] = np.asarray(inputs["bn2_g"], np.float32) / np.sqrt(
        np.float32(1.0) + np.float32(EPS))
    par[:, 7] = inputs["bn2_b"]
    ln1_g = np.asarray(inputs["ln1_g"], np.float32)
    ln1_b = np.asarray(inputs["ln1_b"], np.float32)
    ln2_g = np.asarray(inputs["ln2_g"], np.float32)
    ln2_b = np.asarray(inputs["ln2_b"], np.float32)
    # y = W @ (g*xn + b) + b1 = (W*g) @ xn + (W@b + b1)
    w1f = w1 * ln1_g[None, :]
    b1f = b1 + w1 @ ln1_b
    w2f = w2 * ln2_g[None, :]
    b2f = b2 + w2 @ ln2_b
    par[:, 8] = b1f
    par[:, 9] = b2f
    par_w = np.zeros((128, 256), dtype=bf16)
    par_w[:, 0:128] = w1f.T.astype(bf16)
    par_w[:, 128:256] = w2f.T.astype(bf16)

    x_flat = x.reshape(BP, E)
    in_maps = []
    for c in range(NCORES):
        lo = c * E_SH
        # [bp, e] slice -> pad e to 6272 -> [p, j, bp] -> bf16 [128, 49*512]
        seg_t = np.zeros((E_PAD, BP), dtype=bf16)
        seg_t[:E_SH] = x_flat[:, lo:lo + E_SH].T.astype(bf16)
        x_sh = np.ascontiguousarray(
            seg_t.reshape(NSUB, SUB, BP).transpose(1, 0, 2)
        ).reshape(SUB, NSUB * BP)
        # emb rows -> pad -> [p, j, h] -> bf16 [128, 49*128]
        seg_e = np.zeros((E_PAD, H), dtype=np.float32)
        seg_e[:E_SH] = emb[lo:lo + E_SH, :]
        if c == 0:
            seg_e[0, :] = 0.0   # padding_idx=0
        emb_sh = np.ascontiguousarray(
            seg_e.astype(bf16).reshape(NSUB, SUB, H).transpose(1, 0, 2)
        ).reshape(SUB, NSUB * H)
        in_maps.append({"x": x_sh, "emb": emb_sh, "par": par, "parw": par_w})
    return in_maps


def _run(inputs, trace=False):
    from concourse.bass_utils import run_bass_kernel_spmd

    if "nc" not in _cached:
        _cached["nc"] = _build()
    nc = _cached["nc"]
    in_maps = _prepare_in_maps(inputs)
    res = run_bass_kernel_spmd(
        nc, in_maps, core_ids=list(range(NCORES)), trace=trace)
    out = np.concatenate(
        [np.asarray(res.results[c]["out"]).T for c in range(NCORES)], axis=0)
    return np.ascontiguousarray(out), res.exec_time_ns


def kernel(**inputs) -> np.ndarray:
    out, _ = _run(inputs, trace=False)
    return out
